# revision 61
# baseline (speedup 1.0000x reference)
"""Trainium2 Bass kernel for nn_CHPS_model_20976620273883 (retrieval_knn).

Computes, for x[8192,4096] f32, W[4096,1024] f32, b[1024] f32,
prototypes[1000,1024] f32:

    emb   = x @ W + b
    cos   = normalize(emb) @ normalize(prototypes).T
    out   = (cos - 1) / 0.01            # == 100*cos - 100

Sharding: data-parallel on the batch — each of the 8 NeuronCores gets
1024 rows of x; W / b / prototypes are replicated.  No collectives.

All heavy math runs in fp8e4 (e4m3) with DoubleRow perf mode: each
matmul instruction contracts TWO 128-row k-tiles (2x PE throughput vs
bf16).  Host-side prep packs every operand into the device layout
[128 partitions, k-tiles, free] so the kernel issues only plain copy
DMAs (no transpose DMAs, no xbar mode switches):

  phase 1: embT[d,b] accumulated in PSUM from W'[k-pair, d-tile] (stat)
           and x'[k-pair, b] (moving); W' = 16*W in fp8, x in fp8.
           Two batch-half groups share the 8 PSUM banks; x is packed
           batch-half-major so group 0 is gated by only W + half of x.
  drain:   DVE casts PSUM/16+b -> embT fp8 tiles; squares of embT (fp8)
           on ACT (last two on DVE to avoid an activation-table reload
           on the norm-chain critical path).
  norms:   q[b] = ones.T @ sq (plain fp8 matmuls per d-tile),
           s = 1/sqrt(q*0.1024) = 100/(32*||emb||); row->column for the
           late half via PE transposes of the sqrt row (a 1x1-identity
           transpose per [1,128] slice) + per-column DVE reciprocals,
           for the early half via four SBUF->SBUF column DMAs.
  phase 2: raw[b-tile, p] = embT (stat) @ protoT_n (moving), protoT_n
           host-normalized, *32, padded to 1024 and pre-transposed.
  epilogue: device stores 100*cos = raw*s[b] in fp8 (|100*cos| <= 100
           fits e4m3; RELATIVE fp8 rounding of the small cos values
           beats bf16's 0.5-ulp around -100) split across DVE
           tensor_scalar and ACT Copy(scale); host adds the -100.

Scheduling threads the per-half norm chains (q -> sqrt -> recip ->
scol columns) under the other half's matmuls so phase-2 epilogues
never stall on scale factors, and PSUM tiles cycle one 8-slot ring
whose allocation order matches completion order.
"""

import numpy as np
import ml_dtypes

B, F_IN, D, P = 8192, 4096, 1024, 1000
NCORES = 8
BL = B // NCORES          # 1024 rows per core
KT = F_IN // 128          # 32 contraction tiles (16 DoubleRow pairs)
DT = D // 128             # 8 embedding-dim tiles (4 DoubleRow pairs)
P_PAD = 1024              # prototypes padded to 8 tiles of 128
NB = 512                  # PSUM bank width in fp32
W_SCALE = 16.0            # W*16 has std ~1 in fp8; cast divides back out
P_SCALE = 32.0            # proto_n components ~1/32 -> ~1.0 in fp8

_cache = {}


def _emit(nc, tc, mybir, x_d, w_d, b_d, p_d, o_d):
    f32 = mybir.dt.float32
    bf16 = mybir.dt.bfloat16
    fp8 = mybir.dt.float8e4
    AF = mybir.ActivationFunctionType
    Alu = mybir.AluOpType
    DR = mybir.MatmulPerfMode.DoubleRow

    with (
        tc.tile_pool(name="const", bufs=1) as constp,
        tc.tile_pool(name="wpool", bufs=1) as wpool,
        tc.tile_pool(name="xpool", bufs=1) as xpool,
        tc.tile_pool(name="ppool", bufs=1) as ppool,
        tc.tile_pool(name="embp", bufs=1) as embp,
        tc.tile_pool(name="sqp", bufs=2) as sqp,
        tc.tile_pool(name="sml", bufs=2) as sml,
        tc.tile_pool(name="outp", bufs=8) as outp,
        tc.tile_pool(name="psall", bufs=8, space="PSUM") as psp,
    ):
        # ---- constants -------------------------------------------------
        bcol = constp.tile([128, DT], f32)       # bcol[p,t] = b[t*128+p]
        ones = constp.tile([128, 1], fp8)
        nc.vector.memset(ones[:], 1.0)
        eps1 = constp.tile([1, 1], f32)          # keeps s finite if q == 0
        nc.vector.memset(eps1[:], 1e-20)
        id1 = constp.tile([1, 1], f32)           # 1x1 identity for [1,128]
        nc.vector.memset(id1[:], 1.0)            # PE row->column transposes

        # ---- device-layout operand loads (plain copy DMAs only) --------
        # chunked k loads: few DMAs (HWDGE holds 625ns each), with small
        # leading chunks so the first matmul can start ~2.5us in.  x is
        # packed bc-major on the host: all k-tiles of batch columns 0:512
        # first, then columns 512:1024 — group 0 (bc=0) is gated by only
        # W + half of x (6MB instead of 8MB), and the bc=1 half streams in
        # under group 0's matmuls.
        chunks = [2, 2, 4] + [4] * ((KT - 8) // 4)   # k-tiles per load DMA
        wt = wpool.tile([128, KT, D], fp8)       # wt[p,k,d] = 16*W[k*128+p,d]
        xt = xpool.tile([128, KT, BL], fp8)      # xt[p,k,b] = x[b, k*128+p]
        k0 = 0
        for ci, kc in enumerate(chunks):
            nc.sync.dma_start(
                wt[:, k0:k0 + kc, :],
                w_d.ap()[:, k0 * D:(k0 + kc) * D],
            )
            nc.sync.dma_start(
                xt[:, k0:k0 + kc, 0:NB],
                x_d.ap()[:, k0 * NB:(k0 + kc) * NB],
            )
            if ci == 0:
                nc.sync.dma_start(bcol[:], b_d.ap())
            k0 += kc
        assert k0 == KT
        k0 = 0
        for kc in chunks:
            nc.sync.dma_start(
                xt[:, k0:k0 + kc, NB:BL],
                x_d.ap()[:, KT * NB + k0 * NB:KT * NB + (k0 + kc) * NB],
            )
            k0 += kc
        pt = ppool.tile([128, DT, P_PAD], fp8)   # pt[p,t,j] = 32*proto_n[j,t*128+p]
        nc.sync.dma_start(pt[:], p_d.ap())

        # persistent fp8 embT tiles: embt[:, t, b] = emb[t*128+p, b]
        embt = embp.tile([128, DT, BL], fp8)
        # fp8 squares of embT, one 3D tile per batch half
        sq3 = [sqp.tile([128, DT, NB], fp8, name=f"sq3_{bc}", tag="sq")
               for bc in range(2)]
        s_row = sml.tile([1, BL], f32, name="s_row", bufs=1)
        # separate tiles per half: epilogues for bt 0-3 must not depend on
        # the (later) bc=1 gather DMA via whole-tile dependency tracking
        scols = [constp.tile([128, 4], f32, name=f"scol{bc}")
                 for bc in range(2)]

        # ========== phase 1: embT = W'.T @ x'  (fp8 DoubleRow) ==========
        # bc-major halves.  Group 0 (bc=0) is kk-major so the matmuls stream
        # the k-chunks as the DMAs land; group 1 (bc=1) runs d-tiles in
        # interleaved pairs so banks retire early and in order.  Drains
        # (DVE cast + squares) of the two halves are interleaved so neither
        # engine's in-order queue convoys the other half's chain, and the
        # per-half norm pipelines (q = ones.T@sq on PE -> sqrt on ACT ->
        # recip on DVE -> scol column DMAs) complete under the matmuls.
        def mm1(bank, d, bc, kk):
            nc.tensor.matmul(
                bank[:],
                wt[:, 2 * kk:2 * kk + 2, d * 128:(d + 1) * 128],
                xt[:, 2 * kk:2 * kk + 2, bc * NB:(bc + 1) * NB],
                start=(kk == 0),
                stop=(kk == KT // 2 - 1),
                perf_mode=DR,
            )

        def drain(d, bc, bank, square_on_dve=False):
            eslice = embt[:, d, bc * NB:(bc + 1) * NB]
            # embT = PSUM/16 + b = emb: squares then stay under fp8e4's
            # 240 max (needs |emb| > 15 sigma to overflow)
            nc.vector.tensor_scalar(
                eslice, bank[:], 0.0625, bcol[:, d:d + 1], Alu.mult, Alu.add,
            )
            sq = sq3[bc][:, d, :]
            if square_on_dve:
                # keeps ACT's op stream Square-free after sqrt0 so sqrt1
                # needs no activation-table reload (1.28us on the scol1
                # critical path)
                nc.vector.tensor_tensor(sq, eslice, eslice, Alu.mult)
            else:
                nc.scalar.activation(sq, eslice, AF.Square)

        def qm(qp, d, bc):
            # plain fp8 matmul per d-tile (DoubleRow needs stationary
            # free >= 128: the real ISA rejects a [128,2,1] ldweights)
            nc.tensor.matmul(
                qp[:], ones[:], sq3[bc][:, d, :],
                start=(d == 0), stop=(d == DT - 1),
            )

        def norms_sqrt(bc, qp):
            rt = sml.tile([1, NB], f32, name=f"rt{bc}", tag="rt")
            nc.scalar.activation(rt[:], qp[:], AF.Sqrt, bias=eps1[:],
                                 scale=0.1024)
            return rt

        def norms_s(bc, qp):
            rt = norms_sqrt(bc, qp)
            nc.vector.reciprocal(s_row[:, bc * NB:(bc + 1) * NB], rt[:])

        def scol_cols_dma(bc):
            # row -> column reshape: four SBUF->SBUF DMAs [1,128]->[128,1],
            # one per b-tile, so each epilogue waits only on its own column
            for t in range(4):
                nc.sync.dma_start(
                    scols[bc][:, t:t + 1],
                    s_row[:, bc * NB + t * 128:bc * NB + (t + 1) * 128],
                )

        def scol_cols_pe(bc, rt):
            # row -> column via PE transposes of the sqrt-output row (2
            # cycles each) + tiny per-column DVE reciprocals from PSUM.
            # Reading rt (not s_row) means this only waits on ACT's sqrt,
            # never on a reciprocal queued behind DVE epilogues.
            for t in range(4):
                tp = psp.tile([128, 1], f32, name=f"tp{bc}_{t}", tag="ps")
                nc.tensor.transpose(
                    tp[:], rt[:, t * 128:(t + 1) * 128], id1[:],
                )
                nc.vector.reciprocal(scols[bc][:, t:t + 1], tp[:])

        # group 0: kk-major over all 8 banks
        banks0 = [psp.tile([128, NB], f32, name=f"a0_{d}", tag="ps")
                  for d in range(DT)]
        for kk in range(KT // 2):
            for d in range(DT):
                mm1(banks0[d], d, 0, kk)
        for d in range(4):
            drain(d, 0, banks0[d])

        # group 1 (bc=1), with group-0's remaining drains, both q
        # accumulations, and the bc=0 scalar chain threaded into the stream
        banks1 = [psp.tile([128, NB], f32, name=f"a1_{d}", tag="ps")
                  for d in range(DT)]
        qp1 = psp.tile([1, NB], f32, name="q1", tag="ps")
        qp0 = psp.tile([1, NB], f32, name="q0", tag="ps")
        # d-tiles in interleaved pairs: two active banks give the PE ~3.4us
        # of work per pair so the tail of the bc=1 x stream never stalls it
        for pi in range(DT // 2):
            da, db = 2 * pi, 2 * pi + 1
            for kk in range(KT // 2):
                mm1(banks1[da], da, 1, kk)
                mm1(banks1[db], db, 1, kk)
            drain(da, 1, banks1[da], square_on_dve=(da >= 6))
            drain(db, 1, banks1[db], square_on_dve=(db >= 6))
            if pi < 2:
                drain(2 * pi + 4, 0, banks0[2 * pi + 4])
                drain(2 * pi + 5, 0, banks0[2 * pi + 5])
            if pi == 1:
                qm(qp1, 0, 1)
                qm(qp1, 1, 1)
            if pi == 2:
                for dq in range(DT):
                    qm(qp0, dq, 0)
                qm(qp1, 2, 1)
                qm(qp1, 3, 1)
                norms_s(0, qp0)
                scol_cols_dma(0)
            if pi == 3:
                qm(qp1, 4, 1)
                qm(qp1, 5, 1)
        # q1's last terms land after bt0/bt1's phase-2 matmuls (their
        # squares retire ~1.5us after group 1's last matmul; don't stall
        # the PE)

        # ========== phase 2: raw = embT.T @ protoT  (fp8 DoubleRow) =====
        # output staged in bf16 (host casts back to f32): halves out DMA
        for bt in range(DT):
            sc = scols[bt // 4][:, bt % 4:bt % 4 + 1]
            # fp8 staging: device emits 100*cos (|.|<=100 < e4m3 max 240);
            # the host casts to f32 and subtracts 100.  Quantization adds
            # ~1.1e-3 rel err; halves the output DMA bytes again.
            ot = outp.tile([128, P_PAD], fp8, name="ot")
            for pc in range(2):
                ps2 = psp.tile([128, NB], f32, name="ps2", tag="ps")
                for dd in range(DT // 2):
                    nc.tensor.matmul(
                        ps2[:],
                        embt[:, 2 * dd:2 * dd + 2, bt * 128:(bt + 1) * 128],
                        pt[:, 2 * dd:2 * dd + 2, pc * NB:(pc + 1) * NB],
                        start=(dd == 0),
                        stop=(dd == DT // 2 - 1),
                        perf_mode=DR,
                    )
                # q1's displaced last terms: their squares retire ~1.5us
                # after group 1's last matmul, so they ride inside phase 2.
                # The scalar chain is emitted BEFORE this bt's epilogues so
                # sqrt1/recip1 aren't queued behind them on ACT/DVE.
                if bt == 0 and pc == 0:
                    qm(qp1, 6, 1)
                if bt == 1 and pc == 0:
                    qm(qp1, 7, 1)
                    rt1 = norms_sqrt(1, qp1)
                if bt == 3 and pc == 0:
                    scol_cols_pe(1, rt1)
                # epilogue out = ps2*s[b] - 100 (f32 -> bf16), split across
                # DVE and ACT so neither engine's tail backlog dominates
                if pc == 0:
                    nc.vector.tensor_scalar(
                        ot[:, pc * NB:(pc + 1) * NB], ps2[:],
                        sc, None, Alu.mult,
                    )
                else:
                    nc.scalar.activation(
                        ot[:, pc * NB:(pc + 1) * NB], ps2[:], AF.Copy,
                        bias=0.0, scale=sc,
                    )

            # split output stores across the two hardware-DGE queues
            oeng = nc.scalar if bt % 2 == 0 else nc.sync
            oeng.dma_start(
                o_d.ap()[bt * 128:(bt + 1) * 128, :], ot[:, :P],
            )


def _build(reps=1):
    key = ("mod", reps)
    if key in _cache:
        return _cache[key]
    import concourse.bacc as bacc
    import concourse.mybir as mybir
    import concourse.tile as tile

    nc = bacc.Bacc(
        "TRN2", target_bir_lowering=False, debug=False, num_devices=NCORES
    )
    f32 = mybir.dt.float32
    fp8 = mybir.dt.float8e4
    bf16 = mybir.dt.bfloat16
    x_d = nc.dram_tensor("x", [128, KT * BL], fp8, kind="ExternalInput")
    w_d = nc.dram_tensor("w", [128, KT * D], fp8, kind="ExternalInput")
    b_d = nc.dram_tensor("b", [128, DT], f32, kind="ExternalInput")
    p_d = nc.dram_tensor("protos", [128, DT * P_PAD], fp8, kind="ExternalInput")
    o_d = nc.dram_tensor("out", [BL, P], fp8, kind="ExternalOutput")

    with tile.TileContext(nc) as tc:
        for _ in range(reps):
            _emit(nc, tc, mybir, x_d, w_d, b_d, p_d, o_d)
    nc.compile()
    _cache[key] = nc
    return nc


def _pack_pkf(a2d, ktiles):
    """[ktiles*128, F] -> [128, ktiles*F] with dev[p, k*F+f] = a[k*128+p, f]."""
    k128, F = a2d.shape
    assert k128 == ktiles * 128
    return np.ascontiguousarray(
        a2d.reshape(ktiles, 128, F).transpose(1, 0, 2).reshape(128, ktiles * F)
    )


def _in_maps(inputs):
    fp8 = ml_dtypes.float8_e4m3
    x = np.asarray(inputs["x"], dtype=np.float32)
    W = np.asarray(inputs["W"], dtype=np.float32)
    bb = np.asarray(inputs["b"], dtype=np.float32)
    pp = np.asarray(inputs["prototypes"], dtype=np.float32)

    w_dev = _pack_pkf((W_SCALE * W).astype(fp8), KT)
    b_dev = np.ascontiguousarray(
        bb.reshape(DT, 128).T.astype(np.float32))
    pn = pp / np.maximum(np.linalg.norm(pp, axis=1, keepdims=True), 1e-12)
    pn_pad = np.zeros((P_PAD, D), dtype=np.float32)
    pn_pad[:P] = P_SCALE * pn
    p_dev = _pack_pkf(pn_pad.T.astype(fp8), DT)   # [128, DT*P_PAD]

    x8 = x.astype(fp8)
    maps = []
    for c in range(NCORES):
        blk = x8[c * BL:(c + 1) * BL, :]          # [BL, F_IN]
        # bc-major packing: all k-tiles of batch cols 0:NB, then NB:BL
        halves = [
            _pack_pkf(np.ascontiguousarray(blk[bc * 512:(bc + 1) * 512].T),
                      KT)
            for bc in range(2)
        ]
        x_dev = np.concatenate(halves, axis=1)    # [128, KT*BL]
        maps.append({"x": x_dev, "w": w_dev, "b": b_dev, "protos": p_dev})
    return maps


def kernel(**inputs) -> np.ndarray:
    from concourse import bass_utils

    nc = _build(reps=1)
    in_maps = _in_maps(inputs)
    try:
        res = bass_utils.run_bass_kernel_spmd(
            nc, in_maps, core_ids=list(range(NCORES))
        )
    except Exception:
        # transient axon-session hiccups are recoverable on a second attempt
        res = bass_utils.run_bass_kernel_spmd(
            nc, in_maps, core_ids=list(range(NCORES))
        )
    return np.concatenate(
        [res.results[c]["out"].astype(np.float32) - 100.0
         for c in range(NCORES)],
        axis=0,
    )


# revision 65
# speedup vs baseline: 1.8705x; 1.8705x over previous
"""Trainium2 Bass kernel for nn_CHPS_model_20976620273883 (retrieval_knn).

Computes, for x[8192,4096] f32, W[4096,1024] f32, b[1024] f32,
prototypes[1000,1024] f32:

    emb   = x @ W + b
    cos   = normalize(emb) @ normalize(prototypes).T
    out   = (cos - 1) / 0.01            # == 100*cos - 100

Sharding: data-parallel on the batch — each of the 8 NeuronCores gets
1024 rows of x; W / b / prototypes are replicated.  No collectives.

All heavy math runs in fp8e4 (e4m3) with DoubleRow perf mode: each
matmul instruction contracts TWO 128-row k-tiles (2x PE throughput vs
bf16).  Host-side prep packs every operand into the device layout
[128 partitions, k-tiles, free] so the kernel issues only plain copy
DMAs (no transpose DMAs, no xbar mode switches):

  phase 1: embT[d,b] accumulated in PSUM from W'[k-pair, d-tile] (stat)
           and x'[k-pair, b] (moving); W' = 16*W in fp8, x in fp8.
           Two batch-half groups share the 8 PSUM banks; x is packed
           batch-half-major so group 0 is gated by only W + half of x.
  drain:   DVE casts PSUM/16+b -> embT fp8 tiles; squares of embT (fp8)
           on ACT (last two on DVE to avoid an activation-table reload
           on the norm-chain critical path).
  norms:   q[b] = ones.T @ sq (plain fp8 matmuls per d-tile),
           s = 1/sqrt(q*0.1024) = 100/(32*||emb||); row->column for the
           late half via PE transposes of the sqrt row (a 1x1-identity
           transpose per [1,128] slice) + per-column DVE reciprocals,
           for the early half via four SBUF->SBUF column DMAs.
  phase 2: raw[b-tile, p] = embT (stat) @ protoT_n (moving), protoT_n
           host-normalized, *32, padded to 1024 and pre-transposed.
  epilogue: device stores 100*cos = raw*s[b] in fp8 (|100*cos| <= 100
           fits e4m3; RELATIVE fp8 rounding of the small cos values
           beats bf16's 0.5-ulp around -100) split across DVE
           tensor_scalar and ACT Copy(scale); host adds the -100.

Scheduling threads the per-half norm chains (q -> sqrt -> recip ->
scol columns) under the other half's matmuls so phase-2 epilogues
never stall on scale factors, and PSUM tiles cycle one 8-slot ring
whose allocation order matches completion order.
"""

import numpy as np
import ml_dtypes

B, F_IN, D, P = 8192, 4096, 1024, 1000
NCORES = 8
BL = B // NCORES          # 1024 rows per core
KT = F_IN // 128          # 32 contraction tiles (16 DoubleRow pairs)
DT = D // 128             # 8 embedding-dim tiles (4 DoubleRow pairs)
P_PAD = 1024              # prototypes padded to 8 tiles of 128
NB = 512                  # PSUM bank width in fp32
W_SCALE = 16.0            # W*16 has std ~1 in fp8; cast divides back out
P_SCALE = 32.0            # proto_n components ~1/32 -> ~1.0 in fp8

_cache = {}


def _emit(nc, tc, mybir, x_d, w_d, b_d, p_d, o_d):
    f32 = mybir.dt.float32
    bf16 = mybir.dt.bfloat16
    fp8 = mybir.dt.float8e4
    AF = mybir.ActivationFunctionType
    Alu = mybir.AluOpType
    DR = mybir.MatmulPerfMode.DoubleRow

    with (
        tc.tile_pool(name="const", bufs=1) as constp,
        tc.tile_pool(name="wpool", bufs=1) as wpool,
        tc.tile_pool(name="xpool", bufs=1) as xpool,
        tc.tile_pool(name="ppool", bufs=1) as ppool,
        tc.tile_pool(name="embp", bufs=1) as embp,
        tc.tile_pool(name="sqp", bufs=2) as sqp,
        tc.tile_pool(name="sml", bufs=2) as sml,
        tc.tile_pool(name="outp", bufs=8) as outp,
        tc.tile_pool(name="psall", bufs=8, space="PSUM") as psp,
    ):
        # ---- constants -------------------------------------------------
        bcol = constp.tile([128, DT], f32)       # bcol[p,t] = b[t*128+p]
        # all-ones DoubleRow stationary: every output row is the same
        # partition+pair sum, at the same cost as computing one row
        ones = constp.tile([128, 2, 128], fp8)
        nc.vector.memset(ones[:], 1.0)
        eps1 = constp.tile([1, 1], f32)          # keeps s finite if q == 0
        nc.vector.memset(eps1[:], 1e-20)
        id1 = constp.tile([1, 1], f32)           # 1x1 identity for [1,128]
        nc.vector.memset(id1[:], 1.0)            # PE row->column transposes

        # ---- device-layout operand loads (plain copy DMAs only) --------
        # chunked k loads: few DMAs (HWDGE holds 625ns each), with small
        # leading chunks so the first matmul can start ~2.5us in.  x is
        # packed bc-major on the host: all k-tiles of batch columns 0:512
        # first, then columns 512:1024 — group 0 (bc=0) is gated by only
        # W + half of x (6MB instead of 8MB), and the bc=1 half streams in
        # under group 0's matmuls.
        chunks = [2, 2, 4] + [4] * ((KT - 8) // 4)   # k-tiles per load DMA
        wt = wpool.tile([128, KT, D], fp8)       # wt[p,k,d] = 16*W[k*128+p,d]
        xt = xpool.tile([128, KT, BL], fp8)      # xt[p,k,b] = x[b, k*128+p]
        k0 = 0
        for ci, kc in enumerate(chunks):
            nc.sync.dma_start(
                wt[:, k0:k0 + kc, :],
                w_d.ap()[:, k0 * D:(k0 + kc) * D],
            )
            nc.sync.dma_start(
                xt[:, k0:k0 + kc, 0:NB],
                x_d.ap()[:, k0 * NB:(k0 + kc) * NB],
            )
            if ci == 0:
                nc.sync.dma_start(bcol[:], b_d.ap())
            k0 += kc
        assert k0 == KT
        k0 = 0
        for kc in chunks:
            nc.sync.dma_start(
                xt[:, k0:k0 + kc, NB:BL],
                x_d.ap()[:, KT * NB + k0 * NB:KT * NB + (k0 + kc) * NB],
            )
            k0 += kc
        pt = ppool.tile([128, DT, P_PAD], fp8)   # pt[p,t,j] = 32*proto_n[j,t*128+p]
        nc.sync.dma_start(pt[:], p_d.ap())

        # persistent fp8 embT tiles: embt[:, t, b] = emb[t*128+p, b]
        embt = embp.tile([128, DT, BL], fp8)
        # fp8 squares of embT, one 3D tile per batch half
        sq3 = [sqp.tile([128, DT, NB], fp8, name=f"sq3_{bc}", tag="sq")
               for bc in range(2)]
        s_row = sml.tile([1, BL], f32, name="s_row", bufs=1)
        # separate tiles per half: epilogues for bt 0-3 must not depend on
        # the (later) bc=1 gather DMA via whole-tile dependency tracking
        scols = [constp.tile([128, 4], f32, name=f"scol{bc}")
                 for bc in range(2)]

        # ========== phase 1: embT = W'.T @ x'  (fp8 DoubleRow) ==========
        # bc-major halves.  Group 0 (bc=0) is kk-major so the matmuls stream
        # the k-chunks as the DMAs land; group 1 (bc=1) runs d-tiles in
        # interleaved pairs so banks retire early and in order.  Drains
        # (DVE cast + squares) of the two halves are interleaved so neither
        # engine's in-order queue convoys the other half's chain, and the
        # per-half norm pipelines (q = ones.T@sq on PE -> sqrt on ACT ->
        # recip on DVE -> scol column DMAs) complete under the matmuls.
        def mm1(bank, d, bc, kk):
            nc.tensor.matmul(
                bank[:],
                wt[:, 2 * kk:2 * kk + 2, d * 128:(d + 1) * 128],
                xt[:, 2 * kk:2 * kk + 2, bc * NB:(bc + 1) * NB],
                start=(kk == 0),
                stop=(kk == KT // 2 - 1),
                perf_mode=DR,
            )

        def drain(d, bc, bank, square_on_dve=False):
            eslice = embt[:, d, bc * NB:(bc + 1) * NB]
            # embT = PSUM/16 + b = emb: squares then stay under fp8e4's
            # 240 max (needs |emb| > 15 sigma to overflow)
            nc.vector.tensor_scalar(
                eslice, bank[:], 0.0625, bcol[:, d:d + 1], Alu.mult, Alu.add,
            )
            sq = sq3[bc][:, d, :]
            if square_on_dve:
                # keeps ACT's op stream Square-free after sqrt0 so sqrt1
                # needs no activation-table reload (1.28us on the scol1
                # critical path)
                nc.vector.tensor_tensor(sq, eslice, eslice, Alu.mult)
            else:
                nc.scalar.activation(sq, eslice, AF.Square)

        def qm(qp, dd, bc):
            # fp8 DoubleRow over a d-tile pair; [128,2,128] ones stationary
            # (a [128,2,1] ldweights fails the real ISA check, full-width
            # passes) -> qp rows are 128 copies of the pair sum
            nc.tensor.matmul(
                qp[:], ones[:], sq3[bc][:, 2 * dd:2 * dd + 2, :],
                start=(dd == 0), stop=(dd == DT // 2 - 1),
                perf_mode=DR,
            )

        def norms_sqrt(bc, qp):
            rt = sml.tile([1, NB], f32, name=f"rt{bc}", tag="rt")
            nc.scalar.activation(rt[:], qp[0:1, :], AF.Sqrt, bias=eps1[:],
                                 scale=0.1024)
            return rt

        def norms_s(bc, qp):
            rt = norms_sqrt(bc, qp)
            nc.vector.reciprocal(s_row[:, bc * NB:(bc + 1) * NB], rt[:])

        def scol_cols_dma(bc):
            # row -> column reshape: four SBUF->SBUF DMAs [1,128]->[128,1],
            # one per b-tile, so each epilogue waits only on its own column
            for t in range(4):
                nc.sync.dma_start(
                    scols[bc][:, t:t + 1],
                    s_row[:, bc * NB + t * 128:bc * NB + (t + 1) * 128],
                )

        def scol_cols_pe(bc, rt):
            # row -> column via PE transposes of the sqrt-output row (2
            # cycles each) + tiny per-column DVE reciprocals from PSUM.
            # Reading rt (not s_row) means this only waits on ACT's sqrt,
            # never on a reciprocal queued behind DVE epilogues.
            for t in range(4):
                tp = psp.tile([128, 1], f32, name=f"tp{bc}_{t}", tag="ps")
                nc.tensor.transpose(
                    tp[:], rt[:, t * 128:(t + 1) * 128], id1[:],
                )
                nc.vector.reciprocal(scols[bc][:, t:t + 1], tp[:])

        # group 0: kk-major over all 8 banks
        banks0 = [psp.tile([128, NB], f32, name=f"a0_{d}", tag="ps")
                  for d in range(DT)]
        for kk in range(KT // 2):
            for d in range(DT):
                mm1(banks0[d], d, 0, kk)
        for d in range(4):
            drain(d, 0, banks0[d])

        # group 1 (bc=1), with group-0's remaining drains, both q
        # accumulations, and the bc=0 scalar chain threaded into the stream
        banks1 = [psp.tile([128, NB], f32, name=f"a1_{d}", tag="ps")
                  for d in range(DT)]
        qp1 = psp.tile([128, NB], f32, name="q1", tag="ps")
        qp0 = psp.tile([128, NB], f32, name="q0", tag="ps")
        # d-tiles in interleaved pairs: two active banks give the PE ~3.4us
        # of work per pair so the tail of the bc=1 x stream never stalls it
        for pi in range(DT // 2):
            da, db = 2 * pi, 2 * pi + 1
            for kk in range(KT // 2):
                mm1(banks1[da], da, 1, kk)
                mm1(banks1[db], db, 1, kk)
            drain(da, 1, banks1[da], square_on_dve=(da >= 6))
            drain(db, 1, banks1[db], square_on_dve=(db >= 6))
            if pi < 2:
                drain(2 * pi + 4, 0, banks0[2 * pi + 4])
                drain(2 * pi + 5, 0, banks0[2 * pi + 5])
            if pi == 1:
                qm(qp1, 0, 1)
            if pi == 2:
                for dq in range(DT // 2):
                    qm(qp0, dq, 0)
                qm(qp1, 1, 1)
                norms_s(0, qp0)
                scol_cols_dma(0)
            if pi == 3:
                qm(qp1, 2, 1)
        # q1's last terms land after bt0/bt1's phase-2 matmuls (their
        # squares retire ~1.5us after group 1's last matmul; don't stall
        # the PE)

        # ========== phase 2: raw = embT.T @ protoT  (fp8 DoubleRow) =====
        # output staged in bf16 (host casts back to f32): halves out DMA
        for bt in range(DT):
            sc = scols[bt // 4][:, bt % 4:bt % 4 + 1]
            # fp8 staging: device emits 100*cos (|.|<=100 < e4m3 max 240);
            # the host casts to f32 and subtracts 100.  Quantization adds
            # ~1.1e-3 rel err; halves the output DMA bytes again.
            ot = outp.tile([128, P_PAD], fp8, name="ot")
            for pc in range(2):
                ps2 = psp.tile([128, NB], f32, name="ps2", tag="ps")
                for dd in range(DT // 2):
                    nc.tensor.matmul(
                        ps2[:],
                        embt[:, 2 * dd:2 * dd + 2, bt * 128:(bt + 1) * 128],
                        pt[:, 2 * dd:2 * dd + 2, pc * NB:(pc + 1) * NB],
                        start=(dd == 0),
                        stop=(dd == DT // 2 - 1),
                        perf_mode=DR,
                    )
                # q1's displaced last terms: their squares retire ~1.5us
                # after group 1's last matmul, so they ride inside phase 2.
                # The scalar chain is emitted BEFORE this bt's epilogues so
                # sqrt1/recip1 aren't queued behind them on ACT/DVE.
                if bt == 0 and pc == 0:
                    qm(qp1, 3, 1)   # final pair (d6,d7)
                    rt1 = norms_sqrt(1, qp1)
                if bt == 4 and pc == 0:
                    scol_cols_pe(1, rt1)
                # epilogue: ot = ps2*s[b] (100*cos in fp8), split across
                # DVE and ACT so neither engine's tail backlog dominates
                if pc == 0:
                    nc.vector.tensor_scalar(
                        ot[:, pc * NB:(pc + 1) * NB], ps2[:],
                        sc, None, Alu.mult,
                    )
                else:
                    nc.scalar.activation(
                        ot[:, pc * NB:(pc + 1) * NB], ps2[:], AF.Copy,
                        bias=0.0, scale=sc,
                    )

            # split output stores across the two hardware-DGE queues
            oeng = nc.scalar if bt % 2 == 0 else nc.sync
            oeng.dma_start(
                o_d.ap()[bt * 128:(bt + 1) * 128, :], ot[:, :P],
            )


def _build(reps=1):
    key = ("mod", reps)
    if key in _cache:
        return _cache[key]
    import concourse.bacc as bacc
    import concourse.mybir as mybir
    import concourse.tile as tile

    nc = bacc.Bacc(
        "TRN2", target_bir_lowering=False, debug=False, num_devices=NCORES
    )
    f32 = mybir.dt.float32
    fp8 = mybir.dt.float8e4
    bf16 = mybir.dt.bfloat16
    x_d = nc.dram_tensor("x", [128, KT * BL], fp8, kind="ExternalInput")
    w_d = nc.dram_tensor("w", [128, KT * D], fp8, kind="ExternalInput")
    b_d = nc.dram_tensor("b", [128, DT], f32, kind="ExternalInput")
    p_d = nc.dram_tensor("protos", [128, DT * P_PAD], fp8, kind="ExternalInput")
    o_d = nc.dram_tensor("out", [BL, P], fp8, kind="ExternalOutput")

    with tile.TileContext(nc) as tc:
        for _ in range(reps):
            _emit(nc, tc, mybir, x_d, w_d, b_d, p_d, o_d)
    nc.compile()
    _cache[key] = nc
    return nc


def _pack_pkf(a2d, ktiles):
    """[ktiles*128, F] -> [128, ktiles*F] with dev[p, k*F+f] = a[k*128+p, f]."""
    k128, F = a2d.shape
    assert k128 == ktiles * 128
    return np.ascontiguousarray(
        a2d.reshape(ktiles, 128, F).transpose(1, 0, 2).reshape(128, ktiles * F)
    )


def _in_maps(inputs):
    fp8 = ml_dtypes.float8_e4m3
    x = np.asarray(inputs["x"], dtype=np.float32)
    W = np.asarray(inputs["W"], dtype=np.float32)
    bb = np.asarray(inputs["b"], dtype=np.float32)
    pp = np.asarray(inputs["prototypes"], dtype=np.float32)

    w_dev = _pack_pkf((W_SCALE * W).astype(fp8), KT)
    b_dev = np.ascontiguousarray(
        bb.reshape(DT, 128).T.astype(np.float32))
    pn = pp / np.maximum(np.linalg.norm(pp, axis=1, keepdims=True), 1e-12)
    pn_pad = np.zeros((P_PAD, D), dtype=np.float32)
    pn_pad[:P] = P_SCALE * pn
    p_dev = _pack_pkf(pn_pad.T.astype(fp8), DT)   # [128, DT*P_PAD]

    x8 = x.astype(fp8)
    maps = []
    for c in range(NCORES):
        blk = x8[c * BL:(c + 1) * BL, :]          # [BL, F_IN]
        # bc-major packing: all k-tiles of batch cols 0:NB, then NB:BL
        halves = [
            _pack_pkf(np.ascontiguousarray(blk[bc * 512:(bc + 1) * 512].T),
                      KT)
            for bc in range(2)
        ]
        x_dev = np.concatenate(halves, axis=1)    # [128, KT*BL]
        maps.append({"x": x_dev, "w": w_dev, "b": b_dev, "protos": p_dev})
    return maps


def kernel(**inputs) -> np.ndarray:
    from concourse import bass_utils

    nc = _build(reps=1)
    in_maps = _in_maps(inputs)
    try:
        res = bass_utils.run_bass_kernel_spmd(
            nc, in_maps, core_ids=list(range(NCORES))
        )
    except Exception:
        # transient axon-session hiccups are recoverable on a second attempt
        res = bass_utils.run_bass_kernel_spmd(
            nc, in_maps, core_ids=list(range(NCORES))
        )
    return np.concatenate(
        [res.results[c]["out"].astype(np.float32) - 100.0
         for c in range(NCORES)],
        axis=0,
    )
